# revision 6
# baseline (speedup 1.0000x reference)
"""Trainium2 Bass kernel for CTCDecoderV1-style single-layer attention decoder.

Computes, for x = hid_r [B,S,H], mask [B,S]:
    q = x@Wq + bq ; k = x@Wk + bk ; v = x@Wv + bv
    scores = (q @ k^T) / sqrt(H)   (masked where ~mask -> -1e9)
    attn = softmax(scores, -1)
    ctx = attn @ v
    out = log_softmax(ctx @ W2 + b2, -1)
returns (out [B,S,C], attn [B,S,S]) as float32, matching the jax reference.

Sharding: data-parallel over batch B=32 across 8 NeuronCores (4 batches/core).
Each core runs an identical NEFF on its own batch slice; no collectives.

Per-core kernel layout strategy (S=1024, H=512, C=64, P=128):
  - hid tiles [128,512] are PE-transposed into hidT [H-part, S-free] once.
  - Projections: QT[d,q], KT[d,k] in transposed layout (lhsT = W chunks),
    V[k,d] in natural layout (lhsT = hidT chunks).
  - scores are computed in BOTH layouts from QT/KT (dual matmul instead of
    transposing the softmax output):
      [q,k]: lhsT=QT slice, rhs=KT  -> exp via ScalarE with accum_out giving
             the softmax row-sum for free; normalized attn DMAs straight out.
      [k,q]: lhsT=KT slice, rhs=QT  -> exp -> expT, which is exactly the lhsT
             the ctx matmul needs (ctxT[d,q] = sum_k V[k,d] expT[k,q]).
  - ctx is accumulated transposed (ctxT) so the final W2 matmul needs no
    transpose either; softmax normalization of ctx is folded into the logits
    stage as a per-partition multiply by the cached reciprocal row-sums.
  - log_softmax is fused: reduce_max(negate) -> Exp(bias=-max, accum_out=sum)
    -> Ln -> one tensor_scalar (x - max) - log(sum).

Matmuls run as float32r (full PE rate at free-dim 512; plain fp32 is 4x
slower). Storage stays fp32.

The all-ones mask / all-zero biases of the reference's setup_inputs() are
detected at runtime; the general paths (additive mask via K=1 rank-1 matmul
accumulation, bias adds folded into PSUM evacuations) are only built when
needed, so the common case pays nothing for them.
"""

import sys

for _p in ("/opt/trn_rl_repo",):
    if _p not in sys.path:
        sys.path.insert(0, _p)

import numpy as np

import concourse.bass as bass
import concourse.tile as tile
from concourse import bacc, mybir
from concourse import bass_utils
from concourse.masks import make_identity

F32 = mybir.dt.float32
F32R = mybir.dt.float32r

P = 128          # partitions
S = 1024         # sequence
H = 512          # hidden
C = 64           # classes
B_FULL = 32      # full batch
N_CORES = 8
BL = B_FULL // N_CORES   # local batch per core
HT = H // P      # 4 hidden-dim tiles
SC = S // P      # 8 sequence tiles
QCH = 512        # q chunk (matmul free dim / PSUM bank)
NQC = S // QCH   # 2 q chunks
INV_SQRT_H = 1.0 / float(np.sqrt(H))
MASK_NEG = -1e9


def _mm(ap, dt_):
    """View an fp32 AP with the matmul compute dtype."""
    return ap.bitcast(dt_) if ap.dtype is not dt_ else ap


def build_nc(bl=BL, use_mask=False, use_bias=False, mm_dt=F32R, logit_dt=F32):
    """Build the per-core Bass module. Identical program on every core."""
    nc = bacc.Bacc(
        "TRN2", target_bir_lowering=False, debug=False, num_devices=N_CORES
    )

    hid = nc.dram_tensor("hid", [bl, S, H], F32, kind="ExternalInput").ap()
    wq = nc.dram_tensor("wq", [H, H], F32, kind="ExternalInput").ap()
    wk = nc.dram_tensor("wk", [H, H], F32, kind="ExternalInput").ap()
    wv = nc.dram_tensor("wv", [H, H], F32, kind="ExternalInput").ap()
    w2 = nc.dram_tensor("w2", [H, C], F32, kind="ExternalInput").ap()
    if use_mask:
        # additive mask in raw-score units: (mask-1)*1e9*sqrt(H) prepared host-side
        mbias = nc.dram_tensor("mbias", [bl, S], F32, kind="ExternalInput").ap()
    if use_bias:
        bq = nc.dram_tensor("bq", [H], F32, kind="ExternalInput").ap()
        bk = nc.dram_tensor("bk", [H], F32, kind="ExternalInput").ap()
        bv = nc.dram_tensor("bv", [H], F32, kind="ExternalInput").ap()
        b2 = nc.dram_tensor("b2", [C], F32, kind="ExternalInput").ap()
    out = nc.dram_tensor("out", [bl, S, C], F32, kind="ExternalOutput").ap()
    attn = nc.dram_tensor("attn", [bl, S, S], F32, kind="ExternalOutput").ap()

    with tile.TileContext(nc) as tc:
        with (
            # NOTE: each *tag* gets its own ring of `bufs` slots, so pool cost
            # is (#tags x bufs x tile bytes). Budget ~192KB/partition.
            tc.tile_pool(name="const", bufs=1) as const_pool,      # ~26KB
            tc.tile_pool(name="hid", bufs=10) as hid_pool,         # 20KB
            tc.tile_pool(name="hidT", bufs=1) as hidT_pool,        # 16KB
            tc.tile_pool(name="qkT", bufs=1) as qkT_pool,          # 32KB
            tc.tile_pool(name="v", bufs=SC + 2) as v_pool,         # 20KB
            tc.tile_pool(name="expqk", bufs=SC // 2 + 1) as exp_pool,   # 20KB
            tc.tile_pool(name="expT", bufs=SC + 2) as expT_pool,   # 20KB
            tc.tile_pool(name="ctxT", bufs=1) as ctxT_pool,        # 16KB
            tc.tile_pool(name="rec", bufs=SC + 2) as rec_pool,     # ~2KB
            tc.tile_pool(name="small", bufs=4) as small_pool,      # ~8KB
            tc.tile_pool(name="mm_ps", bufs=4, space="PSUM") as mm_psum,
            tc.tile_pool(name="tr_ps", bufs=2, space="PSUM") as tr_psum,
            tc.tile_pool(name="lg_ps", bufs=2, space="PSUM") as lg_psum,
        ):
            ident = const_pool.tile([P, P], F32, tag="ident")
            make_identity(nc, ident)

            wq_sb, wk_sb, wv_sb = [], [], []
            for ct in range(HT):
                for lst, w, nm in ((wq_sb, wq, "wq"), (wk_sb, wk, "wk"),
                                   (wv_sb, wv, "wv")):
                    t = const_pool.tile([P, H], mm_dt, tag=f"{nm}{ct}")
                    nc.sync.dma_start(t[:], _mm(w[ct * P:(ct + 1) * P, :], mm_dt))
                    lst.append(t)
            w2_sb = []
            for ct in range(HT):
                t = const_pool.tile([P, C], logit_dt, tag=f"w2{ct}")
                nc.sync.dma_start(t[:], _mm(w2[ct * P:(ct + 1) * P, :], logit_dt))
                w2_sb.append(t)

            if use_mask or use_bias:
                ones_row = const_pool.tile([1, P], F32, tag="ones_row")
                nc.vector.memset(ones_row[:], 1.0)

            def bcast_vec(vec_ap, n, nm):
                """[n] DRAM vector -> [P, n] SBUF, replicated across partitions
                via a K=1 rank-1 matmul with a ones column."""
                row = const_pool.tile([1, n], F32, tag=f"{nm}_row")
                nc.sync.dma_start(row[:], vec_ap[None, :])
                ps = mm_psum.tile([P, n], F32, tag="mm")
                nc.tensor.matmul(ps[:], lhsT=ones_row[:], rhs=row[:],
                                 start=True, stop=True)
                bc = const_pool.tile([P, n], F32, tag=f"{nm}_bc")
                nc.any.tensor_copy(bc[:], ps[:])
                return bc

            if use_bias:
                # bq/bk live on the partition dim of QT/KT psum -> [P, HT] stripes
                bq_kp = const_pool.tile([P, HT], F32, tag="bq_kp")
                nc.sync.dma_start(bq_kp[:], bq.rearrange("(t p) -> p t", p=P))
                bk_kp = const_pool.tile([P, HT], F32, tag="bk_kp")
                nc.sync.dma_start(bk_kp[:], bk.rearrange("(t p) -> p t", p=P))
                bv_bc = bcast_vec(bv, H, "bv")      # [P, H]
                b2_bc = bcast_vec(b2, C, "b2")      # [P, C]

            for b in range(bl):
                # ---- load hid and transpose to hidT [H-part, S-free] ----
                hid_tiles = []
                for qt in range(SC):
                    t = hid_pool.tile([P, H], F32, tag="hid")
                    nc.sync.dma_start(t[:], hid[b, qt * P:(qt + 1) * P, :])
                    hid_tiles.append(t)
                hidT = [hidT_pool.tile([P, S], mm_dt, tag=f"hidT{ct}", name=f"hidT{ct}")
                        for ct in range(HT)]
                for qt in range(SC):
                    for ct in range(HT):
                        ps = tr_psum.tile([P, P], F32, tag="tr")
                        nc.tensor.transpose(
                            ps[:], hid_tiles[qt][:, ct * P:(ct + 1) * P], ident[:])
                        nc.any.tensor_copy(
                            hidT[ct][:, qt * P:(qt + 1) * P], ps[:])

                # ---- projections ----
                # QT[d,q] / KT[d,k]: lhsT = W chunk [c,d], rhs = hidT [c,q]
                QT = [qkT_pool.tile([P, S], mm_dt, tag=f"QT{d}", name=f"QT{d}") for d in range(HT)]
                KT = [qkT_pool.tile([P, S], mm_dt, tag=f"KT{d}", name=f"KT{d}") for d in range(HT)]
                for w_sb, dst, bias_kp in (
                    (wq_sb, QT, "bq"), (wk_sb, KT, "bk")
                ):
                    for d in range(HT):
                        for qc in range(NQC):
                            ps = mm_psum.tile([P, QCH], F32, tag="mm")
                            for ct in range(HT):
                                nc.tensor.matmul(
                                    ps[:],
                                    lhsT=_mm(w_sb[ct][:, d * P:(d + 1) * P], mm_dt),
                                    rhs=_mm(hidT[ct][:, qc * QCH:(qc + 1) * QCH], mm_dt),
                                    start=(ct == 0), stop=(ct == HT - 1))
                            dst_sl = dst[d][:, qc * QCH:(qc + 1) * QCH]
                            if use_bias:
                                bkp = bq_kp if bias_kp == "bq" else bk_kp
                                nc.scalar.activation(
                                    dst_sl, ps[:],
                                    mybir.ActivationFunctionType.Identity,
                                    bias=bkp[:, d:d + 1])
                            else:
                                nc.any.tensor_copy(dst_sl, ps[:])
                # V[k,d]: lhsT = hidT chunk [c,k], rhs = Wv [c,d]
                V = []
                for kt in range(SC):
                    ps = mm_psum.tile([P, QCH], F32, tag="mm")
                    for ct in range(HT):
                        nc.tensor.matmul(
                            ps[:],
                            lhsT=_mm(hidT[ct][:, kt * P:(kt + 1) * P], mm_dt),
                            rhs=_mm(wv_sb[ct][:], mm_dt),
                            start=(ct == 0), stop=(ct == HT - 1))
                    vt = v_pool.tile([P, H], mm_dt, tag="V")
                    if use_bias:
                        nc.vector.tensor_add(vt[:], ps[:], bv_bc[:])
                    else:
                        nc.any.tensor_copy(vt[:], ps[:])
                    V.append(vt)

                if use_mask:
                    mb_row = small_pool.tile([1, S], F32, tag="mb_row")
                    nc.sync.dma_start(mb_row[:], mbias[b][None, :])
                    # k-partitioned copy for the [k,q] layout exp bias
                    mb_kp = small_pool.tile([P, SC], F32, tag="mb_kp")
                    nc.sync.dma_start(
                        mb_kp[:], mbias[b].rearrange("(t p) -> p t", p=P))

                recips = []
                ctxT = [ctxT_pool.tile([P, S], logit_dt, tag=f"ctxT{d}", name=f"ctxT{d}")
                        for d in range(HT)]
                for qc in range(NQC):
                    # ---- scoresT -> expT for this q-half ----
                    expT_half = []
                    for kt in range(SC):
                        ps = mm_psum.tile([P, QCH], F32, tag="mm")
                        for d in range(HT):
                            nc.tensor.matmul(
                                ps[:],
                                lhsT=_mm(KT[d][:, kt * P:(kt + 1) * P], mm_dt),
                                rhs=_mm(QT[d][:, qc * QCH:(qc + 1) * QCH], mm_dt),
                                start=(d == 0), stop=(d == HT - 1))
                        et = expT_pool.tile([P, QCH], mm_dt, tag="expT")
                        if use_mask:
                            # mask bias lives on the k (partition) dim here;
                            # convert raw-score units -> post-scale units
                            mb_sc = small_pool.tile([P, 1], F32, tag="mb_sc")
                            nc.vector.tensor_scalar_mul(
                                mb_sc[:], mb_kp[:, kt:kt + 1], INV_SQRT_H)
                            nc.scalar.activation(
                                et[:], ps[:], mybir.ActivationFunctionType.Exp,
                                bias=mb_sc[:], scale=INV_SQRT_H)
                        else:
                            nc.scalar.activation(
                                et[:], ps[:], mybir.ActivationFunctionType.Exp,
                                scale=INV_SQRT_H)
                        expT_half.append(et)

                    # ---- scores [q,k] for the 4 q-tiles of this half ----
                    for qi in range(SC // NQC):
                        qt = qc * (SC // NQC) + qi
                        eq = exp_pool.tile([P, S], F32, tag="expqk")
                        rs = small_pool.tile([P, NQC], F32, tag="rs")
                        for kc in range(NQC):
                            ps = mm_psum.tile([P, QCH], F32, tag="mm")
                            for d in range(HT):
                                nc.tensor.matmul(
                                    ps[:],
                                    lhsT=_mm(QT[d][:, qt * P:(qt + 1) * P], mm_dt),
                                    rhs=_mm(KT[d][:, kc * QCH:(kc + 1) * QCH], mm_dt),
                                    start=(d == 0),
                                    stop=(not use_mask and d == HT - 1))
                            if use_mask:
                                # additive mask (raw-score units) broadcast
                                # over q via rank-1 K=1 matmul accumulation
                                nc.tensor.matmul(
                                    ps[:], lhsT=ones_row[:],
                                    rhs=mb_row[0:1, kc * QCH:(kc + 1) * QCH],
                                    start=False, stop=True)
                            nc.scalar.activation(
                                eq[:, kc * QCH:(kc + 1) * QCH], ps[:],
                                mybir.ActivationFunctionType.Exp,
                                scale=INV_SQRT_H,
                                accum_out=rs[:, kc:kc + 1])
                        rsum = small_pool.tile([P, 1], F32, tag="rsum")
                        nc.vector.tensor_add(rsum[:], rs[:, 0:1], rs[:, 1:2])
                        rec = rec_pool.tile([P, 1], F32, tag="rec")
                        nc.vector.reciprocal(rec[:], rsum[:])
                        recips.append(rec)
                        nc.vector.tensor_scalar_mul(eq[:], eq[:], rec[:])
                        nc.sync.dma_start(attn[b, qt * P:(qt + 1) * P, :], eq[:])

                    # ---- ctxT partial for this q-half ----
                    for d in range(HT):
                        ps = mm_psum.tile([P, QCH], F32, tag="mm")
                        for kt in range(SC):
                            nc.tensor.matmul(
                                ps[:],
                                lhsT=_mm(V[kt][:, d * P:(d + 1) * P], mm_dt),
                                rhs=_mm(expT_half[kt][:], mm_dt),
                                start=(kt == 0), stop=(kt == SC - 1))
                        nc.any.tensor_copy(
                            ctxT[d][:, qc * QCH:(qc + 1) * QCH], ps[:])

                # ---- logits + fused log_softmax ----
                for qt in range(SC):
                    ps = lg_psum.tile([P, C], F32, tag="lg")
                    for d in range(HT):
                        nc.tensor.matmul(
                            ps[:],
                            lhsT=_mm(ctxT[d][:, qt * P:(qt + 1) * P], logit_dt),
                            rhs=_mm(w2_sb[d][:], logit_dt),
                            start=(d == 0), stop=(d == HT - 1))
                    lt = small_pool.tile([P, C], F32, tag="lt")
                    # true logits = ctx_unnorm @ W2 * (1/rowsum) (+ b2)
                    nc.vector.tensor_scalar_mul(lt[:], ps[:], recips[qt][:])
                    if use_bias:
                        nc.vector.tensor_add(lt[:], lt[:], b2_bc[:])
                    nmax = small_pool.tile([P, 1], F32, tag="nmax")
                    nc.vector.reduce_max(
                        nmax[:], lt[:], axis=mybir.AxisListType.X, negate=True)
                    ex = small_pool.tile([P, C], F32, tag="ex")
                    sume = small_pool.tile([P, 1], F32, tag="sume")
                    nc.scalar.activation(
                        ex[:], lt[:], mybir.ActivationFunctionType.Exp,
                        bias=nmax[:], accum_out=sume[:])
                    lse = small_pool.tile([P, 1], F32, tag="lse")
                    nc.scalar.activation(
                        lse[:], sume[:], mybir.ActivationFunctionType.Ln)
                    res = small_pool.tile([P, C], F32, tag="res")
                    nc.vector.tensor_scalar(
                        res[:], lt[:], scalar1=nmax[:], scalar2=lse[:],
                        op0=mybir.AluOpType.add, op1=mybir.AluOpType.subtract)
                    nc.sync.dma_start(out[b, qt * P:(qt + 1) * P, :], res[:])

    nc.compile()
    return nc


_NC_CACHE = {}


def _get_nc(key):
    if key not in _NC_CACHE:
        _NC_CACHE[key] = build_nc(
            bl=BL, use_mask=key[0], use_bias=key[1])
    return _NC_CACHE[key]


def kernel(**inputs):
    hid_r = np.ascontiguousarray(np.asarray(inputs["hid_r"], dtype=np.float32))
    in_mask = np.asarray(inputs["in_mask"])
    Wq = np.ascontiguousarray(np.asarray(inputs["Wq"], dtype=np.float32))
    Wk = np.ascontiguousarray(np.asarray(inputs["Wk"], dtype=np.float32))
    Wv = np.ascontiguousarray(np.asarray(inputs["Wv"], dtype=np.float32))
    W2 = np.ascontiguousarray(np.asarray(inputs["W2"], dtype=np.float32))
    bq = np.asarray(inputs["bq"], dtype=np.float32)
    bk = np.asarray(inputs["bk"], dtype=np.float32)
    bv = np.asarray(inputs["bv"], dtype=np.float32)
    b2 = np.asarray(inputs["b2"], dtype=np.float32)

    use_mask = not bool(np.asarray(in_mask).all())
    use_bias = bool(np.any(bq) or np.any(bk) or np.any(bv) or np.any(b2))
    nc = _get_nc((use_mask, use_bias))

    in_maps = []
    for c in range(N_CORES):
        m = {
            "hid": np.ascontiguousarray(hid_r[c * BL:(c + 1) * BL]),
            "wq": Wq, "wk": Wk, "wv": Wv, "w2": W2,
        }
        if use_mask:
            mb = (np.asarray(in_mask[c * BL:(c + 1) * BL], dtype=np.float32)
                  - 1.0) * (-MASK_NEG) * float(np.sqrt(H))
            m["mbias"] = np.ascontiguousarray(mb.astype(np.float32))
        if use_bias:
            m.update({"bq": bq, "bk": bk, "bv": bv, "b2": b2})
        in_maps.append(m)

    res = bass_utils.run_bass_kernel_spmd(
        nc, in_maps, core_ids=list(range(N_CORES)))
    out = np.concatenate([res.results[c]["out"] for c in range(N_CORES)], axis=0)
    attn = np.concatenate([res.results[c]["attn"] for c in range(N_CORES)], axis=0)
    return out, attn


if __name__ == "__main__":
    rng = np.random.default_rng(0)
    sc = 1.0 / np.sqrt(H)
    inputs = {
        "hid_r": rng.standard_normal((B_FULL, S, H), dtype=np.float32),
        "in_mask": np.ones((B_FULL, S), dtype=bool),
        "Wq": rng.standard_normal((H, H), dtype=np.float32) * sc,
        "bq": np.zeros(H, np.float32),
        "Wk": rng.standard_normal((H, H), dtype=np.float32) * sc,
        "bk": np.zeros(H, np.float32),
        "Wv": rng.standard_normal((H, H), dtype=np.float32) * sc,
        "bv": np.zeros(H, np.float32),
        "W2": rng.standard_normal((H, C), dtype=np.float32) * sc,
        "b2": np.zeros(C, np.float32),
    }
    out, attn = kernel(**inputs)
    print(out.shape, attn.shape, out.dtype, attn.dtype)
